# revision 29
# baseline (speedup 1.0000x reference)
"""GMM e-step (vq_codebook Cluster) kernel for 8 Trainium2 NeuronCores.

Strategy: the output only needs log-probs of each sample's own class
(the one-hot einsum gathers class y[b] before the K-softmax), so we
group samples by class on the host and compute per class slot
(<=64 samples) on device.

Fast path (detected at runtime): when every component covariance is
the SAME isotropic sigma^2*I (true for the module's init covariance
0.5*I), the Mahalanobis term collapses to

    iv*||x - mu||^2 = iv*||x||^2 - 2 x.(iv*mu) + iv*||mu||^2

The quadratic piece -0.5*iv*||x||^2 is a per-sample constant the HOST
folds into the additive constant tile, so the device only computes one
[128x64]-stationary x [128x4] matmul per slot (x . iv*mu) plus the K=4
softmax/logsumexp epilogue.  Total device traffic is ~0.5 MB per core
instead of the 3.7 MB/core precision-matrix stream.

General fallback: host Cholesky/inverse -> per-slot quadratic form
G = x^T A with the slot's 4 A-matrices as one [128,512] moving operand,
lpc = -0.5*rowsum(G*x) + (a.x + cst).  Streams 26 MB of A sharded
across the 8 cores (memory-bound).
"""

import sys
import numpy as np

try:
    import concourse  # noqa: F401
except ImportError:  # pragma: no cover
    for _p in ("/opt/trn_rl_repo", "/root/.axon_site/_ro/trn_rl_repo"):
        if _p not in sys.path:
            sys.path.insert(0, _p)

B, D, C, K = 4096, 128, 100, 4
N_CORES = 8
P = 64              # padded samples per class slot
PAIRS = 7           # slot pairs per core
S = 2 * PAIRS       # class slots per core (14)
TOT = N_CORES * S   # 112 slots >= 100 classes (plus chunk spill room)
LOG2PI = float(np.log(2.0 * np.pi))

# iso-path input layout: [cs | mv | xt], split into three DMA chunks
# so the PE consumes class slots as they land instead of waiting for
# the whole bandwidth-bound stream.  (16-bit input streams were tried:
# they save ~0.4us but push worst-case elementwise error on tiny resp
# entries to 4e-2..3e-1 -- not worth the correctness risk.)
CS0 = 0                             # folded constants (28 cols)
MV0 = PAIRS * K                     # iv*mu per slot (56 cols)
XT0 = MV0 + S * K                   # x^T grouped by slot (896 cols)
NIN = XT0 + S * P
CHUNK_PAIRS = (2, 3, 2)             # pairs per chunk
C1END = XT0 + 2 * CHUNK_PAIRS[0] * P
C2END = C1END + 2 * CHUNK_PAIRS[1] * P

TRACE = False       # test harness flips this to profile

_CACHE = {}


def _epilogue(nc, tc, cpool, mybir, lpc_src_a, lpc_src_b, quad, out_dma_fn,
              NP, tag):
    """K=4 softmax/logsumexp epilogue on [128, NP*K] log-probs.

    lpc = -0.5*a + b if quad else a + b; emits the [128, NP, 9] output
    tile and calls out_dma_fn(out_view).
    """
    f32 = mybir.dt.float32
    mult = mybir.AluOpType.mult
    add = mybir.AluOpType.add
    AF = mybir.ActivationFunctionType
    AX = mybir.AxisListType
    PP = 2 * P
    NC_ = NP * K

    lpc = cpool.tile([PP, NC_], f32, tag=f"lpc{tag}")
    if quad:
        nc.vector.scalar_tensor_tensor(
            out=lpc[:], in0=lpc_src_a, scalar=-0.5, in1=lpc_src_b,
            op0=mult, op1=add)
    else:
        nc.vector.tensor_add(lpc[:], lpc_src_a, lpc_src_b)
    lpc_v = lpc[:].rearrange("p (s k) -> p s k", k=K)

    def bc(tt):  # [PP,NP,1] view -> broadcast [PP,NP,K]
        return tt.broadcast_to([PP, NP, K])

    out_t = cpool.tile([PP, NP * 9], f32, tag=f"out{tag}")
    out_v = out_t[:].rearrange("p (s j) -> p s j", j=9)

    # resp branch head first so the Scalar engine's Exp starts ASAP ...
    mx = cpool.tile([PP, NP], f32, tag=f"mx{tag}")
    nc.vector.tensor_reduce(mx[:], lpc_v, axis=AX.X,
                            op=mybir.AluOpType.max)
    em = cpool.tile([PP, NC_], f32, tag=f"em{tag}")
    em_v = em[:].rearrange("p (s k) -> p s k", k=K)
    nc.vector.tensor_sub(em_v, lpc_v, bc(mx[:].unsqueeze(2)))
    ex = cpool.tile([PP, NC_], f32, tag=f"ex{tag}")
    ex_v = ex[:].rearrange("p (s k) -> p s k", k=K)
    nc.scalar.activation(ex[:], em[:], AF.Exp)

    # ... while the Vector engine runs the scal branch.  The reference's
    # abs() on sum(lpc - min) is an exact no-op (sum of non-negatives).
    mn = cpool.tile([PP, NP], f32, tag=f"mn{tag}")
    nc.vector.tensor_reduce(mn[:], lpc_v, axis=AX.X,
                            op=mybir.AluOpType.min)
    sc0 = cpool.tile([PP, NC_], f32, tag=f"sc0{tag}")
    sc0_v = sc0[:].rearrange("p (s k) -> p s k", k=K)
    nc.vector.tensor_sub(sc0_v, lpc_v, bc(mn[:].unsqueeze(2)))
    ssum = cpool.tile([PP, NP], f32, tag=f"ssum{tag}")
    nc.vector.tensor_reduce(ssum[:], sc0_v, axis=AX.X, op=add)
    rinv = cpool.tile([PP, NP], f32, tag=f"rinv{tag}")
    nc.vector.reciprocal(rinv[:], ssum[:])
    nc.vector.tensor_mul(out_v[:, :, 5:9], sc0_v,
                         bc(rinv[:].unsqueeze(2)))

    se = cpool.tile([PP, NP], f32, tag=f"se{tag}")
    nc.vector.tensor_reduce(se[:], ex_v, axis=AX.X, op=add)
    rse = cpool.tile([PP, NP], f32, tag=f"rse{tag}")
    nc.vector.reciprocal(rse[:], se[:])
    nc.vector.tensor_mul(out_v[:, :, 1:5], ex_v,
                         bc(rse[:].unsqueeze(2)))
    lse = cpool.tile([PP, NP], f32, tag=f"lse{tag}")
    # ln(se * 1/K) = ln(se) - log K  (fold uniform log-pi in)
    nc.scalar.activation(lse[:], se[:], AF.Ln, scale=1.0 / K)
    # log_probs_sum = lse + mx, written straight into output col 0
    nc.vector.tensor_add(out_v[:, :, 0:1], lse[:].unsqueeze(2),
                         mx[:].unsqueeze(2))
    nc.vector.tensor_sub(out_v[:, :, 5:9], out_v[:, :, 5:9],
                         bc(out_v[:, :, 0:1]))
    out_dma_fn(out_t)


def _patch_act_tables():
    """Make the act-table-load pass use the combined exp+ln table set.

    The greedy pass otherwise picks `exp_and_others` for Exp and
    `natural_log` for Ln; the ACT engine holds one table set at a time,
    so the Exp->Ln switch in the epilogue costs a 1.3us ACT_TABLE_LOAD
    on the critical path.  Stripping Exp/Ln from every other set forces
    `natural_log_exp_and_others` (one load, no switches).
    """
    import concourse.bacc as bacc_mod
    import concourse.mybir as mybir
    if getattr(bacc_mod, "_act_tables_patched", False):
        return
    orig = bacc_mod.get_activation_tables

    def patched(arch):
        tables = orig(arch)
        E, L = (mybir.ActivationFunctionType.Exp,
                mybir.ActivationFunctionType.Ln)
        both = [n for n, fs in tables.items() if E in fs and L in fs]
        if both:
            for n, fs in tables.items():
                if n != both[0]:
                    fs.discard(E)
                    fs.discard(L)
        return tables

    bacc_mod.get_activation_tables = patched
    bacc_mod._act_tables_patched = True


def _warm_act(nc, tc, cpool, mybir):
    # Warm the ACT transcendental tables during startup dead time so
    # the epilogue's Exp/Ln don't stall on 1.3us ACT_TABLE_LOADs.
    f32 = mybir.dt.float32
    AF = mybir.ActivationFunctionType
    warm = cpool.tile([1, 4], f32)
    warm2 = cpool.tile([1, 4], f32)
    with tc.high_priority():
        nc.vector.memset(warm[:], 1.0)
        nc.scalar.activation(warm2[:], warm[:], AF.Exp)
        nc.scalar.activation(warm2[:], warm[:], AF.Ln, scale=1.0 / K)


def _build_module_iso():
    _patch_act_tables()
    import concourse.bacc as bacc
    import concourse.bass as bass
    import concourse.mybir as mybir
    import concourse.tile as tile

    f32 = mybir.dt.float32
    nc = bacc.Bacc("TRN2", target_bir_lowering=False, debug=False,
                   num_devices=N_CORES)

    biga_d = nc.dram_tensor("biga", [2 * P, C1END], f32,
                            kind="ExternalInput")
    bigb_d = nc.dram_tensor("bigb", [2 * P, C2END - C1END], f32,
                            kind="ExternalInput")
    bigc_d = nc.dram_tensor("bigc", [2 * P, NIN - C2END], f32,
                            kind="ExternalInput")
    out_d = nc.dram_tensor("out", [2 * P, PAIRS * 9], f32,
                           kind="ExternalOutput")
    PP = 2 * P

    with tile.TileContext(nc) as tc:
        with (
            tc.tile_pool(name="const", bufs=1) as cpool,
            tc.tile_pool(name="dpsum", bufs=1,
                         space=bass.MemorySpace.PSUM) as dpool,
        ):
            # All input chunks go on the Sync hardware-DGE ring in
            # consumption order: the DMA channels are FIFO, so chunk 1's
            # descriptors must be written first or the PE stalls on the
            # wrong chunk.  (GpSimd's ring is software-DGE and ~6us
            # slower; don't route bulk input through it.)
            c1 = cpool.tile([PP, C1END], f32)
            nc.sync.dma_start(c1[:], biga_d.ap())
            c2 = cpool.tile([PP, C2END - C1END], f32)
            nc.sync.dma_start(c2[:], bigb_d.ap())
            c3 = cpool.tile([PP, NIN - C2END], f32)
            nc.sync.dma_start(c3[:], bigc_d.ap())
            _warm_act(nc, tc, cpool, mybir)

            # Warm the PE pipeline during startup dead time so the first
            # real matmul doesn't pay the cold-pipeline penalty.
            wpe = cpool.tile([PP, P + K], f32)
            wps = dpool.tile([P, K], f32)
            with tc.high_priority():
                nc.vector.memset(wpe[:], 0.0)
                nc.tensor.matmul(wps[:], wpe[:, 0:P], wpe[:, P:P + K],
                                 start=True, stop=True)

            def xs(s):  # slot s stationary [D, P]
                if s < 2 * CHUNK_PAIRS[0]:
                    return c1[:, XT0 + s * P:XT0 + (s + 1) * P]
                if s < 2 * (CHUNK_PAIRS[0] + CHUNK_PAIRS[1]):
                    o = (s - 2 * CHUNK_PAIRS[0]) * P
                    return c2[:, o:o + P]
                o = (s - 2 * (CHUNK_PAIRS[0] + CHUNK_PAIRS[1])) * P
                return c3[:, o:o + P]

            dot = dpool.tile([PP, PAIRS * K], f32)
            for j in range(PAIRS):
                sA, sB = 2 * j, 2 * j + 1
                nc.tensor.matmul(
                    dot[0:P, j * K:(j + 1) * K], xs(sA),
                    c1[:, MV0 + sA * K:MV0 + (sA + 1) * K],
                    start=True, stop=True)
                nc.tensor.matmul(
                    dot[P:PP, j * K:(j + 1) * K], xs(sB),
                    c1[:, MV0 + sB * K:MV0 + (sB + 1) * K],
                    start=True, stop=True)

            def out_dma(out_t):
                # Scalar is idle after the epilogue's Ln; issuing from it
                # keeps Sync out of the tail
                nc.scalar.dma_start(out_d.ap(), out_t[:])

            _epilogue(nc, tc, cpool, mybir,
                      dot[:], c1[:, CS0:CS0 + PAIRS * K], False, out_dma,
                      PAIRS, "i")

    nc.compile()
    return nc


def _build_module_general():
    _patch_act_tables()
    import concourse.bacc as bacc
    import concourse.bass as bass
    import concourse.mybir as mybir
    import concourse.tile as tile

    f32 = mybir.dt.float32
    nc = bacc.Bacc("TRN2", target_bir_lowering=False, debug=False,
                   num_devices=N_CORES)

    xt_d = nc.dram_tensor("xt", [D, S * P], f32, kind="ExternalInput")
    xr_d = nc.dram_tensor("xr", [2 * P, PAIRS * D], f32, kind="ExternalInput")
    a_d = nc.dram_tensor("arhs", [S, D, K * D], f32, kind="ExternalInput")
    av_d = nc.dram_tensor("avec", [D, S * K], f32, kind="ExternalInput")
    cs_d = nc.dram_tensor("cstb", [2 * P, PAIRS * K], f32,
                          kind="ExternalInput")
    out_d = nc.dram_tensor("out", [PAIRS, 2 * P, 9], f32,
                           kind="ExternalOutput")

    mult = mybir.AluOpType.mult
    add = mybir.AluOpType.add
    AX = mybir.AxisListType
    PP = 2 * P  # 128 partitions

    with tile.TileContext(nc) as tc:
        with (
            tc.tile_pool(name="const", bufs=1) as cpool,
            tc.tile_pool(name="astream", bufs=8) as apool,
            tc.tile_pool(name="scr", bufs=4) as spool,
            tc.tile_pool(name="gpsum", bufs=6,
                         space=bass.MemorySpace.PSUM) as gpool,
            tc.tile_pool(name="dpsum", bufs=1,
                         space=bass.MemorySpace.PSUM) as dpool,
        ):
            xt = cpool.tile([D, S * P], f32)
            nc.sync.dma_start(xt[:], xt_d.ap())
            av = cpool.tile([D, S * K], f32)
            nc.sync.dma_start(av[:], av_d.ap())
            xr = cpool.tile([PP, PAIRS * D], f32)
            cs = cpool.tile([PP, PAIRS * K], f32)

            _warm_act(nc, tc, cpool, mybir)

            halves = [(0, 4), (4, PAIRS)]
            acc_h = {}
            dot_h = {}
            for hi, (j0, j1) in enumerate(halves):
                acc_h[hi] = cpool.tile([PP, (j1 - j0) * K], f32,
                                       name=f"acc{hi}", tag=f"acc{hi}")
                dot_h[hi] = dpool.tile([PP, (j1 - j0) * K], f32,
                                       name=f"dot{hi}", tag=f"dot{hi}")

            for j in range(PAIRS):
                hi = 0 if j < halves[0][1] else 1
                j0 = halves[hi][0]
                sA, sB = 2 * j, 2 * j + 1
                atA = apool.tile([D, K * D], f32, tag="at")
                nc.sync.dma_start(atA[:], a_d.ap()[sA])
                atB = apool.tile([D, K * D], f32, tag="at")
                nc.sync.dma_start(atB[:], a_d.ap()[sB])
                if j == 0:
                    nc.sync.dma_start(xr[:], xr_d.ap())
                elif j == 1:
                    nc.sync.dma_start(cs[:], cs_d.ap())
                sxA = xt[:, sA * P:(sA + 1) * P]
                sxB = xt[:, sB * P:(sB + 1) * P]
                g = gpool.tile([PP, K * D], f32)
                nc.tensor.matmul(g[0:P, :], sxA, atA[:],
                                 start=True, stop=True)
                nc.tensor.matmul(g[P:PP, :], sxB, atB[:],
                                 start=True, stop=True)
                dcol = (j - j0) * K
                nc.tensor.matmul(dot_h[hi][0:P, dcol:dcol + K], sxA,
                                 av[:, sA * K:(sA + 1) * K],
                                 start=True, stop=True)
                nc.tensor.matmul(dot_h[hi][P:PP, dcol:dcol + K], sxB,
                                 av[:, sB * K:(sB + 1) * K],
                                 start=True, stop=True)
                mt = spool.tile([PP, K * D], f32)
                xr_b = (xr[:, j * D:(j + 1) * D]
                        .unsqueeze(1).broadcast_to([PP, K, D]))
                nc.vector.tensor_tensor(
                    mt[:].rearrange("p (k d) -> p k d", k=K),
                    g[:].rearrange("p (k d) -> p k d", k=K),
                    xr_b, op=mult)
                nc.vector.tensor_reduce(
                    acc_h[hi][:, dcol:dcol + K],
                    mt[:].rearrange("p (k d) -> p k d", k=K),
                    axis=AX.X, op=add)

            # ---- epilogue per half, overlapping the other half's PE work
            for hi, (j0, j1) in enumerate(halves):
                dc = cpool.tile([PP, (j1 - j0) * K], f32, tag=f"dch{hi}")
                nc.vector.tensor_add(dc[:], dot_h[hi][:],
                                     cs[:, j0 * K:j1 * K])

                def out_dma(out_t, j0=j0, j1=j1):
                    out_v = out_t[:].rearrange("p (s j) -> p s j", j=9)
                    nc.sync.dma_start(
                        out_d.ap()[j0:j1].rearrange("s p j -> p s j"), out_v)

                _epilogue(nc, tc, cpool, mybir,
                          acc_h[hi][:], dc[:], True, out_dma,
                          j1 - j0, f"h{hi}")

    nc.compile()
    return nc


def _group_slots(y):
    slots = []  # (class_id, sample_indices)
    for c in range(C):
        idx = np.nonzero(y == c)[0]
        for j in range(0, len(idx), P):
            slots.append((c, idx[j:j + P]))
    assert len(slots) <= TOT, f"{len(slots)} slots > {TOT}"
    return slots


def _run(nc, in_maps):
    from concourse.bass_utils import run_bass_kernel_spmd
    trace = TRACE
    if trace:
        _install_ntff_hook()
    res = run_bass_kernel_spmd(nc, in_maps, core_ids=list(range(N_CORES)),
                               trace=trace)
    if trace and res.exec_time_ns is not None:
        print(f"HW exec time: {res.exec_time_ns} ns "
              f"(mean {res.mean_exec_time_ns} ns)")
        kernel.last_exec_time_ns = res.exec_time_ns
        kernel.last_results = res
    return res


def _scatter_out(res, slots):
    out = np.empty((B, 9), np.float32)
    for g, (c, idx) in enumerate(slots):
        core, s = divmod(g, S)
        pj, half = divmod(s, 2)
        r = res.results[core]["out"]
        if r.ndim == 3:       # general path: [PAIRS, 128, 9]
            rows = r[pj]
        else:                 # iso path: [128, PAIRS*9]
            rows = r[:, pj * 9:(pj + 1) * 9]
        out[idx] = rows[half * P:half * P + len(idx), :]
    return out


def _kernel_iso(x, y, sigma2, mu):
    """All covariances are the same sigma2 * I."""
    iv = 1.0 / float(sigma2)
    mu64 = mu.astype(np.float64)                  # [CK, D]
    avec = (iv * mu64).astype(np.float32).reshape(C, K, D)
    # cst = -0.5*(D*log2pi + iv*||mu||^2) - 0.5*D*log(sigma2)
    q = iv * np.sum(mu64 * mu64, axis=1)
    cst = (-0.5 * (q + D * LOG2PI)
           - 0.5 * D * np.log(float(sigma2))).reshape(C, K)
    # per-sample quadratic term, folded into the constant tile
    gq = -0.5 * iv * np.sum(x.astype(np.float64) ** 2, axis=1)  # [B]

    slots = _group_slots(y)
    big_all = np.zeros((N_CORES, 2 * P, NIN), np.float32)
    for g, (c, idx) in enumerate(slots):
        core, s = divmod(g, S)
        pj, half = divmod(s, 2)
        n = len(idx)
        big_all[core, :, XT0 + s * P:XT0 + s * P + n] = x[idx].T
        big_all[core, :, MV0 + s * K:MV0 + (s + 1) * K] = avec[c].T
        big_all[core, half * P:half * P + n,
                CS0 + pj * K:CS0 + (pj + 1) * K] = \
            cst[c][None, :] + gq[idx, None]

    if "iso" not in _CACHE:
        _CACHE["iso"] = _build_module_iso()
    in_maps = [
        {"biga": np.ascontiguousarray(big_all[i, :, :C1END]),
         "bigb": np.ascontiguousarray(big_all[i, :, C1END:C2END]),
         "bigc": np.ascontiguousarray(big_all[i, :, C2END:])}
        for i in range(N_CORES)
    ]
    res = _run(_CACHE["iso"], in_maps)
    return _scatter_out(res, slots)


def _kernel_general(x, y, mu, cov):
    # ---- host factorization (tiny: 400 x 128^3) ----
    cov64 = cov.astype(np.float64)
    L = np.linalg.cholesky(cov64)
    logdet = np.sum(np.log(np.diagonal(L, axis1=-2, axis2=-1)), axis=-1)
    A = np.linalg.inv(cov64)
    A = (A + A.transpose(0, 2, 1)) * 0.5
    a_vec = np.einsum('nij,nj->ni', A, mu.astype(np.float64))
    q = np.einsum('ni,ni->n', mu.astype(np.float64), a_vec)
    cst = (-0.5 * (q + D * LOG2PI) - logdet).astype(np.float32)
    A = A.astype(np.float32).reshape(C, K, D, D)
    a_vec = a_vec.astype(np.float32).reshape(C, K, D)
    cst = cst.reshape(C, K)

    slots = _group_slots(y)
    xt_all = np.zeros((N_CORES, D, S * P), np.float32)
    xr_all = np.zeros((N_CORES, 2 * P, PAIRS * D), np.float32)
    a_all = np.zeros((N_CORES, S, D, K * D), np.float32)
    av_all = np.zeros((N_CORES, D, S * K), np.float32)
    cs_all = np.zeros((N_CORES, 2 * P, PAIRS * K), np.float32)

    for g, (c, idx) in enumerate(slots):
        core, s = divmod(g, S)
        pj, half = divmod(s, 2)
        n = len(idx)
        xs = x[idx]
        xt_all[core, :, s * P:s * P + n] = xs.T
        xr_all[core, half * P:half * P + n, pj * D:(pj + 1) * D] = xs
        a_all[core, s] = A[c].transpose(1, 0, 2).reshape(D, K * D)
        av_all[core, :, s * K:(s + 1) * K] = a_vec[c].T
        cs_all[core, half * P:(half + 1) * P, pj * K:(pj + 1) * K] = \
            cst[c][None, :]

    if "gen" not in _CACHE:
        _CACHE["gen"] = _build_module_general()
    in_maps = [
        {"xt": xt_all[i], "xr": xr_all[i], "arhs": a_all[i],
         "avec": av_all[i], "cstb": cs_all[i]}
        for i in range(N_CORES)
    ]
    res = _run(_CACHE["gen"], in_maps)
    return _scatter_out(res, slots)


def kernel(x, y, class_mu, class_cov):
    x = np.ascontiguousarray(np.asarray(x, dtype=np.float32))
    y = np.asarray(y).astype(np.int64)
    mu = np.asarray(class_mu, dtype=np.float32).reshape(C * K, D)
    cov = np.asarray(class_cov, dtype=np.float32).reshape(C * K, D, D)

    # Fast path: every component covariance is the same sigma^2 * I
    # (exact check; true for the module's init covariance 0.5*I).
    sigma2 = cov[0, 0, 0]
    if sigma2 > 0 and np.all(
            cov == sigma2 * np.eye(D, dtype=np.float32)):
        return _kernel_iso(x, y, sigma2, mu)
    return _kernel_general(x, y, mu, cov)


def _install_ntff_hook():
    import types
    import antenv  # noqa: F401
    if "antenv.axon_hooks" in sys.modules:
        return
    hooks = types.ModuleType("antenv.axon_hooks")
    hooks._hook = None
    hooks.set_axon_ntff_profile_hook = lambda h: setattr(hooks, "_hook", h)
    hooks.get_axon_ntff_profile_hook = lambda: hooks._hook
    sys.modules["antenv.axon_hooks"] = hooks
    try:
        from trn_agent_boot.trn_boot import _ntff_profile_via_ctypes
        hooks.set_axon_ntff_profile_hook(
            _ntff_profile_via_ctypes("/opt/axon/libaxon_pjrt.so"))
        import concourse.bass_utils as bu
        bu.upload_artifacts = lambda d: d
    except Exception:
        pass


# revision 33
# speedup vs baseline: 1.0495x; 1.0495x over previous
"""GMM e-step (vq_codebook Cluster) kernel for 8 Trainium2 NeuronCores.

Strategy: the output only needs log-probs of each sample's own class
(the one-hot einsum gathers class y[b] before the K-softmax), so we
group samples by class on the host and compute per class slot
(<=64 samples) on device.

Fast path (detected at runtime): when every component covariance is
the SAME isotropic sigma^2*I (true for the module's init covariance
0.5*I), the Mahalanobis term collapses to

    iv*||x - mu||^2 = iv*||x||^2 - 2 x.(iv*mu) + iv*||mu||^2

The quadratic piece -0.5*iv*||x||^2 is a per-sample constant the HOST
folds into the additive constant tile, so the device only computes one
[128x64]-stationary x [128x4] matmul per slot (x . iv*mu) plus the K=4
softmax/logsumexp epilogue.  Total device traffic is ~0.5 MB per core
instead of the 3.7 MB/core precision-matrix stream.

General fallback: host Cholesky/inverse -> per-slot quadratic form
G = x^T A with the slot's 4 A-matrices as one [128,512] moving operand,
lpc = -0.5*rowsum(G*x) + (a.x + cst).  Streams 26 MB of A sharded
across the 8 cores (memory-bound).
"""

import sys
import numpy as np

try:
    import concourse  # noqa: F401
except ImportError:  # pragma: no cover
    for _p in ("/opt/trn_rl_repo", "/root/.axon_site/_ro/trn_rl_repo"):
        if _p not in sys.path:
            sys.path.insert(0, _p)

B, D, C, K = 4096, 128, 100, 4
N_CORES = 8
P = 64              # padded samples per class slot
PAIRS = 7           # slot pairs per core
S = 2 * PAIRS       # class slots per core (14)
TOT = N_CORES * S   # 112 slots >= 100 classes (plus chunk spill room)
LOG2PI = float(np.log(2.0 * np.pi))

# iso-path input layout: [cs | mv | xt], split into three DMA chunks
# so the PE consumes class slots as they land instead of waiting for
# the whole bandwidth-bound stream.  (16-bit input streams were tried:
# they save ~0.4us but push worst-case elementwise error on tiny resp
# entries to 4e-2..3e-1 -- not worth the correctness risk.)
CS0 = 0                             # folded constants (28 cols)
MV0 = PAIRS * K                     # iv*mu per slot (56 cols)
XT0 = MV0 + S * K                   # x^T grouped by slot (896 cols)
NIN = XT0 + S * P
CHUNK_PAIRS = (2, 3, 2)             # pairs per chunk
C1END = XT0 + 2 * CHUNK_PAIRS[0] * P
C2END = C1END + 2 * CHUNK_PAIRS[1] * P

TRACE = False       # test harness flips this to profile

_CACHE = {}


def _epilogue(nc, tc, cpool, mybir, lpc_src_a, lpc_src_b, quad, out_dma_fn,
              NP, tag):
    """K=4 softmax/logsumexp epilogue on [128, NP*K] log-probs.

    lpc = -0.5*a + b if quad else a + b; emits the [128, NP, 9] output
    tile and calls out_dma_fn(out_view).
    """
    f32 = mybir.dt.float32
    mult = mybir.AluOpType.mult
    add = mybir.AluOpType.add
    AF = mybir.ActivationFunctionType
    AX = mybir.AxisListType
    PP = 2 * P
    NC_ = NP * K

    # Scratch is packed into few tiles (grouped by writing engine so no
    # false cross-engine serialization): every allocated tile costs a
    # semaphore, and the end-of-program teardown clears + barrier-waits
    # each one at ~0.1-0.3us apiece.
    lpc = cpool.tile([PP, NC_], f32, tag=f"lpc{tag}")
    if quad:
        nc.vector.scalar_tensor_tensor(
            out=lpc[:], in0=lpc_src_a, scalar=-0.5, in1=lpc_src_b,
            op0=mult, op1=add)
    else:
        nc.vector.tensor_add(lpc[:], lpc_src_a, lpc_src_b)
    lpc_v = lpc[:].rearrange("p (s k) -> p s k", k=K)

    def bc(tt):  # [PP,NP,1] view -> broadcast [PP,NP,K]
        return tt.broadcast_to([PP, NP, K])

    out_t = cpool.tile([PP, NP * 9], f32, tag=f"out{tag}")
    out_v = out_t[:].rearrange("p (s j) -> p s j", j=9)

    vw = cpool.tile([PP, NC_ * 2 + NP * 6], f32, tag=f"vw{tag}")
    em_v = vw[:, 0:NC_].rearrange("p (s k) -> p s k", k=K)
    sc0_v = vw[:, NC_:2 * NC_].rearrange("p (s k) -> p s k", k=K)
    o = 2 * NC_
    mx = vw[:, o:o + NP]
    mn = vw[:, o + NP:o + 2 * NP]
    ssum = vw[:, o + 2 * NP:o + 3 * NP]
    rinv = vw[:, o + 3 * NP:o + 4 * NP]
    se = vw[:, o + 4 * NP:o + 5 * NP]
    rse = vw[:, o + 5 * NP:o + 6 * NP]
    sw = cpool.tile([PP, NC_ + NP], f32, tag=f"sw{tag}")
    ex_v = sw[:, 0:NC_].rearrange("p (s k) -> p s k", k=K)
    lse = sw[:, NC_:NC_ + NP]

    # resp branch head first so the Scalar engine's Exp starts ASAP ...
    nc.vector.tensor_reduce(mx, lpc_v, axis=AX.X,
                            op=mybir.AluOpType.max)
    nc.vector.tensor_sub(em_v, lpc_v, bc(mx.unsqueeze(2)))
    nc.scalar.activation(sw[:, 0:NC_], vw[:, 0:NC_], AF.Exp)

    # ... while the Vector engine runs the scal branch.  The reference's
    # abs() on sum(lpc - min) is an exact no-op (sum of non-negatives).
    nc.vector.tensor_reduce(mn, lpc_v, axis=AX.X,
                            op=mybir.AluOpType.min)
    nc.vector.tensor_sub(sc0_v, lpc_v, bc(mn.unsqueeze(2)))
    nc.vector.tensor_reduce(ssum, sc0_v, axis=AX.X, op=add)
    nc.vector.reciprocal(rinv, ssum)
    nc.vector.tensor_mul(out_v[:, :, 5:9], sc0_v,
                         bc(rinv.unsqueeze(2)))

    nc.vector.tensor_reduce(se, ex_v, axis=AX.X, op=add)
    nc.vector.reciprocal(rse, se)
    nc.vector.tensor_mul(out_v[:, :, 1:5], ex_v,
                         bc(rse.unsqueeze(2)))
    # ln(se * 1/K) = ln(se) - log K  (fold uniform log-pi in)
    nc.scalar.activation(lse, se, AF.Ln, scale=1.0 / K)
    # log_probs_sum = lse + mx, written straight into output col 0
    nc.vector.tensor_add(out_v[:, :, 0:1], lse.unsqueeze(2),
                         mx.unsqueeze(2))
    nc.vector.tensor_sub(out_v[:, :, 5:9], out_v[:, :, 5:9],
                         bc(out_v[:, :, 0:1]))
    out_dma_fn(out_t)


def _patch_act_tables():
    """Make the act-table-load pass use the combined exp+ln table set.

    The greedy pass otherwise picks `exp_and_others` for Exp and
    `natural_log` for Ln; the ACT engine holds one table set at a time,
    so the Exp->Ln switch in the epilogue costs a 1.3us ACT_TABLE_LOAD
    on the critical path.  Stripping Exp/Ln from every other set forces
    `natural_log_exp_and_others` (one load, no switches).
    """
    import concourse.bacc as bacc_mod
    import concourse.mybir as mybir
    if getattr(bacc_mod, "_act_tables_patched", False):
        return
    orig = bacc_mod.get_activation_tables

    def patched(arch):
        tables = orig(arch)
        E, L = (mybir.ActivationFunctionType.Exp,
                mybir.ActivationFunctionType.Ln)
        both = [n for n, fs in tables.items() if E in fs and L in fs]
        if both:
            for n, fs in tables.items():
                if n != both[0]:
                    fs.discard(E)
                    fs.discard(L)
        return tables

    bacc_mod.get_activation_tables = patched
    bacc_mod._act_tables_patched = True


def _warm_act(nc, tc, warm, warm2, mybir):
    # Warm the ACT transcendental tables during startup dead time so
    # the epilogue's Exp/Ln don't stall on 1.3us ACT_TABLE_LOADs.
    AF = mybir.ActivationFunctionType
    with tc.high_priority():
        nc.scalar.activation(warm2, warm, AF.Exp)
        nc.scalar.activation(warm2, warm, AF.Ln, scale=1.0 / K)


def _build_module_iso():
    _patch_act_tables()
    import concourse.bacc as bacc
    import concourse.bass as bass
    import concourse.mybir as mybir
    import concourse.tile as tile

    f32 = mybir.dt.float32
    nc = bacc.Bacc("TRN2", target_bir_lowering=False, debug=False,
                   num_devices=N_CORES)

    biga_d = nc.dram_tensor("biga", [2 * P, C1END], f32,
                            kind="ExternalInput")
    bigb_d = nc.dram_tensor("bigb", [2 * P, C2END - C1END], f32,
                            kind="ExternalInput")
    bigc_d = nc.dram_tensor("bigc", [2 * P, NIN - C2END], f32,
                            kind="ExternalInput")
    out_d = nc.dram_tensor("out", [2 * P, PAIRS * 9], f32,
                           kind="ExternalOutput")
    PP = 2 * P

    with tile.TileContext(nc) as tc:
        with (
            tc.tile_pool(name="const", bufs=1) as cpool,
            tc.tile_pool(name="dpsum", bufs=1,
                         space=bass.MemorySpace.PSUM) as dpool,
        ):
            # All input chunks go on the Sync hardware-DGE ring in
            # consumption order: the DMA channels are FIFO, so chunk 1's
            # descriptors must be written first or the PE stalls on the
            # wrong chunk.  (GpSimd's ring is software-DGE and ~6us
            # slower; don't route bulk input through it.)
            c1 = cpool.tile([PP, C1END], f32)
            nc.sync.dma_start(c1[:], biga_d.ap())
            c2 = cpool.tile([PP, C2END - C1END], f32)
            nc.sync.dma_start(c2[:], bigb_d.ap())
            c3 = cpool.tile([PP, NIN - C2END], f32)
            nc.sync.dma_start(c3[:], bigc_d.ap())

            # Warm the PE pipeline during startup dead time so the first
            # real matmul doesn't pay the cold-pipeline penalty; the same
            # tile feeds the ACT table warm.
            wpe = cpool.tile([PP, P + 2 * K], f32)
            wps = dpool.tile([P, K], f32)
            with tc.high_priority():
                nc.vector.memset(wpe[:], 1.0)
                nc.tensor.matmul(wps[:], wpe[:, 0:P], wpe[:, P:P + K],
                                 start=True, stop=True)
            _warm_act(nc, tc, wpe[0:1, 0:K], wpe[0:1, P + K:P + 2 * K],
                      mybir)

            def xs(s):  # slot s stationary [D, P]
                if s < 2 * CHUNK_PAIRS[0]:
                    return c1[:, XT0 + s * P:XT0 + (s + 1) * P]
                if s < 2 * (CHUNK_PAIRS[0] + CHUNK_PAIRS[1]):
                    o = (s - 2 * CHUNK_PAIRS[0]) * P
                    return c2[:, o:o + P]
                o = (s - 2 * (CHUNK_PAIRS[0] + CHUNK_PAIRS[1])) * P
                return c3[:, o:o + P]

            dot = dpool.tile([PP, PAIRS * K], f32)
            for j in range(PAIRS):
                sA, sB = 2 * j, 2 * j + 1
                nc.tensor.matmul(
                    dot[0:P, j * K:(j + 1) * K], xs(sA),
                    c1[:, MV0 + sA * K:MV0 + (sA + 1) * K],
                    start=True, stop=True)
                nc.tensor.matmul(
                    dot[P:PP, j * K:(j + 1) * K], xs(sB),
                    c1[:, MV0 + sB * K:MV0 + (sB + 1) * K],
                    start=True, stop=True)

            def out_dma(out_t):
                # Scalar is idle after the epilogue's Ln; issuing from it
                # keeps Sync out of the tail
                nc.scalar.dma_start(out_d.ap(), out_t[:])

            _epilogue(nc, tc, cpool, mybir,
                      dot[:], c1[:, CS0:CS0 + PAIRS * K], False, out_dma,
                      PAIRS, "i")

    nc.compile()
    return nc


def _build_module_general():
    _patch_act_tables()
    import concourse.bacc as bacc
    import concourse.bass as bass
    import concourse.mybir as mybir
    import concourse.tile as tile

    f32 = mybir.dt.float32
    nc = bacc.Bacc("TRN2", target_bir_lowering=False, debug=False,
                   num_devices=N_CORES)

    xt_d = nc.dram_tensor("xt", [D, S * P], f32, kind="ExternalInput")
    xr_d = nc.dram_tensor("xr", [2 * P, PAIRS * D], f32, kind="ExternalInput")
    a_d = nc.dram_tensor("arhs", [S, D, K * D], f32, kind="ExternalInput")
    av_d = nc.dram_tensor("avec", [D, S * K], f32, kind="ExternalInput")
    cs_d = nc.dram_tensor("cstb", [2 * P, PAIRS * K], f32,
                          kind="ExternalInput")
    out_d = nc.dram_tensor("out", [PAIRS, 2 * P, 9], f32,
                           kind="ExternalOutput")

    mult = mybir.AluOpType.mult
    add = mybir.AluOpType.add
    AX = mybir.AxisListType
    PP = 2 * P  # 128 partitions

    with tile.TileContext(nc) as tc:
        with (
            tc.tile_pool(name="const", bufs=1) as cpool,
            tc.tile_pool(name="astream", bufs=8) as apool,
            tc.tile_pool(name="scr", bufs=4) as spool,
            tc.tile_pool(name="gpsum", bufs=6,
                         space=bass.MemorySpace.PSUM) as gpool,
            tc.tile_pool(name="dpsum", bufs=1,
                         space=bass.MemorySpace.PSUM) as dpool,
        ):
            xt = cpool.tile([D, S * P], f32)
            nc.sync.dma_start(xt[:], xt_d.ap())
            av = cpool.tile([D, S * K], f32)
            nc.sync.dma_start(av[:], av_d.ap())
            xr = cpool.tile([PP, PAIRS * D], f32)
            cs = cpool.tile([PP, PAIRS * K], f32)

            warm = cpool.tile([1, 2 * K], f32)
            with tc.high_priority():
                nc.vector.memset(warm[:], 1.0)
            _warm_act(nc, tc, warm[0:1, 0:K], warm[0:1, K:2 * K], mybir)

            halves = [(0, 4), (4, PAIRS)]
            acc_h = {}
            dot_h = {}
            for hi, (j0, j1) in enumerate(halves):
                acc_h[hi] = cpool.tile([PP, (j1 - j0) * K], f32,
                                       name=f"acc{hi}", tag=f"acc{hi}")
                dot_h[hi] = dpool.tile([PP, (j1 - j0) * K], f32,
                                       name=f"dot{hi}", tag=f"dot{hi}")

            for j in range(PAIRS):
                hi = 0 if j < halves[0][1] else 1
                j0 = halves[hi][0]
                sA, sB = 2 * j, 2 * j + 1
                atA = apool.tile([D, K * D], f32, tag="at")
                nc.sync.dma_start(atA[:], a_d.ap()[sA])
                atB = apool.tile([D, K * D], f32, tag="at")
                nc.sync.dma_start(atB[:], a_d.ap()[sB])
                if j == 0:
                    nc.sync.dma_start(xr[:], xr_d.ap())
                elif j == 1:
                    nc.sync.dma_start(cs[:], cs_d.ap())
                sxA = xt[:, sA * P:(sA + 1) * P]
                sxB = xt[:, sB * P:(sB + 1) * P]
                g = gpool.tile([PP, K * D], f32)
                nc.tensor.matmul(g[0:P, :], sxA, atA[:],
                                 start=True, stop=True)
                nc.tensor.matmul(g[P:PP, :], sxB, atB[:],
                                 start=True, stop=True)
                dcol = (j - j0) * K
                nc.tensor.matmul(dot_h[hi][0:P, dcol:dcol + K], sxA,
                                 av[:, sA * K:(sA + 1) * K],
                                 start=True, stop=True)
                nc.tensor.matmul(dot_h[hi][P:PP, dcol:dcol + K], sxB,
                                 av[:, sB * K:(sB + 1) * K],
                                 start=True, stop=True)
                mt = spool.tile([PP, K * D], f32)
                xr_b = (xr[:, j * D:(j + 1) * D]
                        .unsqueeze(1).broadcast_to([PP, K, D]))
                nc.vector.tensor_tensor(
                    mt[:].rearrange("p (k d) -> p k d", k=K),
                    g[:].rearrange("p (k d) -> p k d", k=K),
                    xr_b, op=mult)
                nc.vector.tensor_reduce(
                    acc_h[hi][:, dcol:dcol + K],
                    mt[:].rearrange("p (k d) -> p k d", k=K),
                    axis=AX.X, op=add)

            # ---- epilogue per half, overlapping the other half's PE work
            for hi, (j0, j1) in enumerate(halves):
                dc = cpool.tile([PP, (j1 - j0) * K], f32, tag=f"dch{hi}")
                nc.vector.tensor_add(dc[:], dot_h[hi][:],
                                     cs[:, j0 * K:j1 * K])

                def out_dma(out_t, j0=j0, j1=j1):
                    out_v = out_t[:].rearrange("p (s j) -> p s j", j=9)
                    nc.sync.dma_start(
                        out_d.ap()[j0:j1].rearrange("s p j -> p s j"), out_v)

                _epilogue(nc, tc, cpool, mybir,
                          acc_h[hi][:], dc[:], True, out_dma,
                          j1 - j0, f"h{hi}")

    nc.compile()
    return nc


def _group_slots(y):
    slots = []  # (class_id, sample_indices)
    for c in range(C):
        idx = np.nonzero(y == c)[0]
        for j in range(0, len(idx), P):
            slots.append((c, idx[j:j + P]))
    assert len(slots) <= TOT, f"{len(slots)} slots > {TOT}"
    return slots


def _run(nc, in_maps):
    from concourse.bass_utils import run_bass_kernel_spmd
    trace = TRACE
    if trace:
        _install_ntff_hook()
    res = run_bass_kernel_spmd(nc, in_maps, core_ids=list(range(N_CORES)),
                               trace=trace)
    if trace and res.exec_time_ns is not None:
        print(f"HW exec time: {res.exec_time_ns} ns "
              f"(mean {res.mean_exec_time_ns} ns)")
        kernel.last_exec_time_ns = res.exec_time_ns
        kernel.last_results = res
    return res


def _scatter_out(res, slots):
    out = np.empty((B, 9), np.float32)
    for g, (c, idx) in enumerate(slots):
        core, s = divmod(g, S)
        pj, half = divmod(s, 2)
        r = res.results[core]["out"]
        if r.ndim == 3:       # general path: [PAIRS, 128, 9]
            rows = r[pj]
        else:                 # iso path: [128, PAIRS*9]
            rows = r[:, pj * 9:(pj + 1) * 9]
        out[idx] = rows[half * P:half * P + len(idx), :]
    return out


def _kernel_iso(x, y, sigma2, mu):
    """All covariances are the same sigma2 * I."""
    iv = 1.0 / float(sigma2)
    mu64 = mu.astype(np.float64)                  # [CK, D]
    avec = (iv * mu64).astype(np.float32).reshape(C, K, D)
    # cst = -0.5*(D*log2pi + iv*||mu||^2) - 0.5*D*log(sigma2)
    q = iv * np.sum(mu64 * mu64, axis=1)
    cst = (-0.5 * (q + D * LOG2PI)
           - 0.5 * D * np.log(float(sigma2))).reshape(C, K)
    # per-sample quadratic term, folded into the constant tile
    gq = -0.5 * iv * np.sum(x.astype(np.float64) ** 2, axis=1)  # [B]

    slots = _group_slots(y)
    big_all = np.zeros((N_CORES, 2 * P, NIN), np.float32)
    for g, (c, idx) in enumerate(slots):
        core, s = divmod(g, S)
        pj, half = divmod(s, 2)
        n = len(idx)
        big_all[core, :, XT0 + s * P:XT0 + s * P + n] = x[idx].T
        big_all[core, :, MV0 + s * K:MV0 + (s + 1) * K] = avec[c].T
        big_all[core, half * P:half * P + n,
                CS0 + pj * K:CS0 + (pj + 1) * K] = \
            cst[c][None, :] + gq[idx, None]

    if "iso" not in _CACHE:
        _CACHE["iso"] = _build_module_iso()
    in_maps = [
        {"biga": np.ascontiguousarray(big_all[i, :, :C1END]),
         "bigb": np.ascontiguousarray(big_all[i, :, C1END:C2END]),
         "bigc": np.ascontiguousarray(big_all[i, :, C2END:])}
        for i in range(N_CORES)
    ]
    res = _run(_CACHE["iso"], in_maps)
    return _scatter_out(res, slots)


def _kernel_general(x, y, mu, cov):
    # ---- host factorization (tiny: 400 x 128^3) ----
    cov64 = cov.astype(np.float64)
    L = np.linalg.cholesky(cov64)
    logdet = np.sum(np.log(np.diagonal(L, axis1=-2, axis2=-1)), axis=-1)
    A = np.linalg.inv(cov64)
    A = (A + A.transpose(0, 2, 1)) * 0.5
    a_vec = np.einsum('nij,nj->ni', A, mu.astype(np.float64))
    q = np.einsum('ni,ni->n', mu.astype(np.float64), a_vec)
    cst = (-0.5 * (q + D * LOG2PI) - logdet).astype(np.float32)
    A = A.astype(np.float32).reshape(C, K, D, D)
    a_vec = a_vec.astype(np.float32).reshape(C, K, D)
    cst = cst.reshape(C, K)

    slots = _group_slots(y)
    xt_all = np.zeros((N_CORES, D, S * P), np.float32)
    xr_all = np.zeros((N_CORES, 2 * P, PAIRS * D), np.float32)
    a_all = np.zeros((N_CORES, S, D, K * D), np.float32)
    av_all = np.zeros((N_CORES, D, S * K), np.float32)
    cs_all = np.zeros((N_CORES, 2 * P, PAIRS * K), np.float32)

    for g, (c, idx) in enumerate(slots):
        core, s = divmod(g, S)
        pj, half = divmod(s, 2)
        n = len(idx)
        xs = x[idx]
        xt_all[core, :, s * P:s * P + n] = xs.T
        xr_all[core, half * P:half * P + n, pj * D:(pj + 1) * D] = xs
        a_all[core, s] = A[c].transpose(1, 0, 2).reshape(D, K * D)
        av_all[core, :, s * K:(s + 1) * K] = a_vec[c].T
        cs_all[core, half * P:(half + 1) * P, pj * K:(pj + 1) * K] = \
            cst[c][None, :]

    if "gen" not in _CACHE:
        _CACHE["gen"] = _build_module_general()
    in_maps = [
        {"xt": xt_all[i], "xr": xr_all[i], "arhs": a_all[i],
         "avec": av_all[i], "cstb": cs_all[i]}
        for i in range(N_CORES)
    ]
    res = _run(_CACHE["gen"], in_maps)
    return _scatter_out(res, slots)


def kernel(x, y, class_mu, class_cov):
    x = np.ascontiguousarray(np.asarray(x, dtype=np.float32))
    y = np.asarray(y).astype(np.int64)
    mu = np.asarray(class_mu, dtype=np.float32).reshape(C * K, D)
    cov = np.asarray(class_cov, dtype=np.float32).reshape(C * K, D, D)

    # Fast path: every component covariance is the same sigma^2 * I
    # (exact check; true for the module's init covariance 0.5*I).
    sigma2 = cov[0, 0, 0]
    if sigma2 > 0 and np.all(
            cov == sigma2 * np.eye(D, dtype=np.float32)):
        return _kernel_iso(x, y, sigma2, mu)
    return _kernel_general(x, y, mu, cov)


def _install_ntff_hook():
    import types
    import antenv  # noqa: F401
    if "antenv.axon_hooks" in sys.modules:
        return
    hooks = types.ModuleType("antenv.axon_hooks")
    hooks._hook = None
    hooks.set_axon_ntff_profile_hook = lambda h: setattr(hooks, "_hook", h)
    hooks.get_axon_ntff_profile_hook = lambda: hooks._hook
    sys.modules["antenv.axon_hooks"] = hooks
    try:
        from trn_agent_boot.trn_boot import _ntff_profile_via_ctypes
        hooks.set_axon_ntff_profile_hook(
            _ntff_profile_via_ctypes("/opt/axon/libaxon_pjrt.so"))
        import concourse.bass_utils as bu
        bu.upload_artifacts = lambda d: d
    except Exception:
        pass
